# revision 19
# baseline (speedup 1.0000x reference)
"""GCN layer (normalized adjacency @ features -> linear -> relu) on 8 TRN2 NeuronCores.

Strategy (row-sharded, 1D node partition), v2:
  - Host shards adj by rows (P=1024 rows/core), adds the identity diagonal,
    TRANSPOSES the shard and stores it as fp8e4 (0/1/2 values are exact in
    e4m3, so this is lossless): 8MB/core of HBM traffic instead of 32MB, and
    no on-chip transpose at all. Tiles are pre-shuffled on the host so every
    DMA reads 4KB contiguous per partition line.
  - Rowsums: ones-vector matmuls on the PE against the fp8 tiles as they
    land (contraction over the partition dim = adjacency columns). The adj
    columns are split into two i-groups so gather#0 (own rows 0..511) fires
    at ~50% of streaming and hides under the rest.
  - Two AllGathers of the per-group rowsums; d = rsqrt via Sqrt + reciprocal
    + one Newton step. Own-row d is computed locally (no gather needed).
  - Main matmul: out_pre.T accumulated over 64 j-stripes with mixed-dtype
    matmuls (stationary = d-scaled bf16 features, moving = fp8 adj.T), then
    a small fp32 matmul with W.T, per-row d scale, bias, relu; per-stripe
    output DMA. Host concatenates the 8 [P,128] outputs.
"""

import numpy as np
import ml_dtypes

import concourse.bass as bass
import concourse.bacc as bacc
import concourse.mybir as mybir
import concourse.tile as tile
from concourse.bass_utils import run_bass_kernel_spmd

F32 = mybir.dt.float32
BF16 = mybir.dt.bfloat16
FP8 = mybir.dt.float8e4

N_FULL = 8192
F_DIM = 128
NUM_CORES = 8


def build_kernel(P=1024, N=8192, F=128, num_cores=8):
    """Build the SPMD Bass program. P = rows per core; N = total nodes."""
    assert P % 256 == 0 and N % 1024 == 0 and F == 128
    n_st = P // 128            # own-row stripes per core (8)
    n_jb = N // 128            # j-stripes globally (64)
    GB = N // 1024             # at-tile count per pass (j blocks of 1024)
    GC = P // 2                # columns per i-group (512)
    spg = n_st // 2            # own-row stripes per group (4)
    n_ts = N // n_st           # feature stripes (64) == n_jb

    nc = bacc.Bacc("TRN2", target_bir_lowering=False, debug=False,
                   num_devices=num_cores)

    # host-preshuffled adj.T: [g, b, p, jj, i] with j = b*1024 + jj*128 + p,
    # i = g*GC + i_local. Per (g,b,p): 8*GC bytes contiguous.
    at_h = nc.declare_dram_parameter("at8", [2, GB, 128, 8, GC], FP8,
                                     isOutput=False)
    ones8_h = nc.declare_dram_parameter("ones8", [128, 2, 128], FP8,
                                        isOutput=False)
    feat_h = nc.declare_dram_parameter("feat16", [128, n_jb, F], BF16,
                                       isOutput=False)
    w_h = nc.declare_dram_parameter("w", [F, F], F32, isOutput=False)
    bias_h = nc.declare_dram_parameter("bias_b", [128, F], F32, isOutput=False)
    eye32_h = nc.declare_dram_parameter("eye32", [128, 128], F32, isOutput=False)
    out_h = nc.declare_dram_parameter("out", [P, F], F32, isOutput=True)

    r_loc = nc.dram_tensor("r_local", [1, P], F32)
    r_ful = nc.dram_tensor("r_full", [num_cores, P], F32,
                           addr_space="Shared")


    at_ap = at_h.ap().rearrange("g b p jj i -> p g b jj i")
    out_ap = out_h.ap().rearrange("(s p) f -> p s f", p=128)

    with tile.TileContext(nc) as tc:
        with tc.tile_pool(name="const", bufs=1) as cpool, \
             tc.tile_pool(name="atp", bufs=1) as atp, \
             tc.tile_pool(name="p2", bufs=1) as p2, \
             tc.tile_pool(name="psR", bufs=1, space="PSUM") as psR, \
             tc.tile_pool(name="psM", bufs=1, space="PSUM") as psM, \
             tc.tile_pool(name="psT", bufs=1, space="PSUM") as psT:

            eye32 = cpool.tile([128, 128], F32)
            nc.scalar.dma_start(eye32, eye32_h[:])
            w_sb = cpool.tile([128, F], F32)
            nc.scalar.dma_start(w_sb, w_h[:])
            bias_bc = cpool.tile([128, F], F32)
            nc.scalar.dma_start(bias_bc, bias_h[:])
            feat16 = cpool.tile([128, n_jb, F], BF16)

            ones8 = cpool.tile([128, 2, 128], FP8)
            nc.scalar.dma_start(ones8, ones8_h[:])
            # pre-warm the Sqrt activation table set so the d = rsqrt(r)
            # chains don't pay the ~2.7us table load
            warm = cpool.tile([1, 1], F32)
            nc.scalar.activation(warm, eye32[0:1, 0:1],
                                 mybir.ActivationFunctionType.Sqrt)

            # PSUM residents
            pm = [psM.tile([128, GC], F32, tag=f"pm{h}", name=f"pm{h}")
                  for h in range(2)]
            down = psT.tile([128, n_st], F32, tag="down")

            at_tiles = [[None] * GB, [None] * GB]
            rs = cpool.tile([1, P], F32, name="rs")

            # ---- phase 1: stream adj.T, accumulate rowsums on the PE ----
            # rowsums via fp8 DoubleRow (2 j-stripes per matmul)
            for g in range(2):
                rsum = psR.tile([128, GC], F32, tag="rsum", name="rsum")
                for b in range(GB):
                    at = atp.tile([128, 8, GC], FP8, tag=f"at{g}{b}",
                                  name="at")
                    nc.sync.dma_start(at, at_ap[:, g, b])
                    at_tiles[g][b] = at
                    for jj in range(0, 8, 2):
                        nc.tensor.matmul(
                            rsum, lhsT=ones8, rhs=at[:, jj:jj + 2, :],
                            perf_mode=mybir.MatmulPerfMode.DoubleRow,
                            start=(b == 0 and jj == 0),
                            stop=(b == GB - 1 and jj == 6))
                nc.scalar.copy(rs[0:1, g * GC:(g + 1) * GC], rsum[0:1, :])
                if g == 0:
                    # feature load deferred: keeps pass-0 HBM bandwidth and
                    # the scalar ring free for the rowsum flush
                    nc.scalar.dma_start(feat16, feat_h[:])
            # single flush + single AllGather of all 1024 own-row sums
            nc.scalar.dma_start(r_loc[:], rs)
            nc.gpsimd.collective_compute(
                "AllGather", mybir.AluOpType.bypass,
                replica_groups=[list(range(num_cores))],
                ins=[r_loc[:].opt()],
                outs=[r_ful[:].opt()],
            )

            # ---- phase 2: d, df, main matmul, epilogue ----
            def rsqrt_newton(r_in, width, nm):
                sq = p2.tile([128, width], F32, tag=f"sq{nm}", name=f"sq{nm}")
                nc.scalar.activation(sq, r_in,
                                     mybir.ActivationFunctionType.Sqrt)
                y0 = p2.tile([128, width], F32, tag=f"y0{nm}", name=f"y0{nm}")
                nc.vector.reciprocal(y0, sq)
                yy = p2.tile([128, width], F32, tag=f"yy{nm}", name=f"yy{nm}")
                nc.vector.tensor_mul(yy, y0, y0)
                ryy = p2.tile([128, width], F32, tag=f"ry{nm}", name=f"ryy{nm}")
                nc.vector.tensor_mul(ryy, yy, r_in)
                corr = p2.tile([128, width], F32, tag=f"co{nm}", name=f"corr{nm}")
                nc.vector.tensor_scalar(out=corr, in0=ryy, scalar1=-0.5,
                                        scalar2=1.5,
                                        op0=mybir.AluOpType.mult,
                                        op1=mybir.AluOpType.add)
                d = p2.tile([128, width], F32, tag=f"d{nm}", name=f"d{nm}")
                nc.vector.tensor_mul(d, y0, corr)
                return d

            # W.T (stationary for the per-stripe linear) + own-row d: no
            # gather dependency, fills the gather wait window
            for s in range(n_st):
                nc.tensor.matmul(
                    down[:, s:s + 1],
                    lhsT=rs[0:1, s * 128:(s + 1) * 128],
                    rhs=eye32[0:1, 0:1], start=True, stop=True)
            pw = psT.tile([128, F], F32, tag="pp", bufs=2, name="pw")
            nc.tensor.matmul(pw, lhsT=w_sb, rhs=eye32, start=True, stop=True)
            wt_sb = cpool.tile([128, F], F32)
            nc.scalar.copy(wt_sb, pw)
            d_own = rsqrt_newton(down, n_st, "o")

            opre = p2.tile([128, P], F32)
            out_sb = p2.tile([128, n_st, F], F32)

            # d for all 64 stripes from the gathered rowsums
            rf = p2.tile([n_jb, 128], F32, name="rf")
            nc.sync.dma_start(rf, r_ful.ap().rearrange(
                "c (s p) -> (c s) p", p=128))
            prT = psT.tile([128, n_jb], F32, tag="prT", name="prT")
            nc.tensor.matmul(prT, lhsT=rf, rhs=eye32[0:n_jb, 0:n_jb],
                             start=True, stop=True)
            d_all = rsqrt_newton(prT, n_jb, "a")

            # all 64 dfs, fed from two engines; every tile stays live so the
            # producers can sprint ahead of the PE (no WAR throttling)
            dfs = []
            for t in range(n_jb):
                df = p2.tile([128, F], BF16, tag="df", bufs=n_jb, name="df")
                dcol = d_all[:, t:t + 1]
                if t % 2 == 0:
                    nc.scalar.mul(df, feat16[:, t, :], dcol)
                else:
                    nc.vector.tensor_scalar(
                        out=df, in0=feat16[:, t, :], scalar1=dcol,
                        scalar2=None, op0=mybir.AluOpType.mult)
                dfs.append(df)

            def finish_chunk(hc):
                nc.scalar.copy(opre[:, hc * GC:(hc + 1) * GC], pm[hc])
                for s in range(hc * spg, (hc + 1) * spg):
                    sl = s - hc * spg
                    p2m = psT.tile([128, F], F32, tag="pp", bufs=2,
                                   name="p2m")
                    nc.tensor.matmul(
                        p2m, lhsT=opre[:, (hc * spg + sl) * 128:
                                       (hc * spg + sl + 1) * 128],
                        rhs=wt_sb, start=True, stop=True)
                    epi = p2.tile([128, F], F32, tag="epi", bufs=2,
                                  name="epi")
                    nc.vector.scalar_tensor_tensor(
                        out=epi, in0=p2m, scalar=d_own[:, s:s + 1],
                        in1=bias_bc, op0=mybir.AluOpType.mult,
                        op1=mybir.AluOpType.add)
                    nc.vector.tensor_scalar_max(out_sb[:, s, :], epi, 0.0)
                    nc.scalar.dma_start(out_ap[:, s, :], out_sb[:, s, :])

            # main matmul, chunk-major: chunk 0's epilogue overlaps chunk 1
            for hc in range(2):
                for t in range(n_jb):
                    nc.tensor.matmul(
                        pm[hc], lhsT=dfs[t],
                        rhs=at_tiles[hc][t // 8][:, t % 8, :],
                        start=(t == 0), stop=(t == n_jb - 1))
                finish_chunk(hc)

    nc.compile()
    return nc


def make_in_maps(adj, features, W, b, P, num_cores):
    """Shard + pre-transpose + fp8-encode inputs on the host."""
    adj = np.asarray(adj, dtype=np.float32)
    features = np.asarray(features, dtype=np.float32)
    W = np.asarray(W, dtype=np.float32)
    b = np.asarray(b, dtype=np.float32)
    N = adj.shape[0]
    GB = N // 1024
    GC = P // 2
    eye32 = np.eye(128, dtype=np.float32)
    bias_b = np.broadcast_to(b[None, :], (128, b.shape[0])).copy()
    feat16 = np.ascontiguousarray(
        features.astype(ml_dtypes.bfloat16)
        .reshape(N // 128, 128, features.shape[1]).transpose(1, 0, 2))
    in_maps = []
    idx = np.arange(P)
    for c in range(num_cores):
        shT = adj[c * P:(c + 1) * P, :].T.astype(ml_dtypes.float8_e4m3)
        # add the identity diagonal (0/1 -> 1/2; exact in e4m3)
        diag = shT[c * P + idx, idx].astype(np.float32) + 1.0
        shT[c * P + idx, idx] = diag.astype(ml_dtypes.float8_e4m3)
        # [N, P] -> [g, b, p, jj, i]
        at8 = np.ascontiguousarray(
            shT.reshape(GB, 8, 128, 2, GC).transpose(3, 0, 2, 1, 4))
        in_maps.append({
            "at8": at8,
            "ones8": np.ones((128, 2, 128), dtype=ml_dtypes.float8_e4m3),
            "feat16": feat16,
            "w": W,
            "bias_b": bias_b,
            "eye32": eye32,
        })
    return in_maps


_NC_CACHE = {}


def get_nc(P=N_FULL // NUM_CORES, N=N_FULL, F=F_DIM, num_cores=NUM_CORES):
    key = (P, N, F, num_cores)
    if key not in _NC_CACHE:
        _NC_CACHE[key] = build_kernel(P, N, F, num_cores)
    return _NC_CACHE[key]


def kernel(**inputs):
    adj = np.asarray(inputs["adj"], dtype=np.float32)
    features = np.asarray(inputs["features"], dtype=np.float32)
    W = np.asarray(inputs["W"], dtype=np.float32)
    b = np.asarray(inputs["b"], dtype=np.float32)
    n = adj.shape[0]
    P = n // NUM_CORES
    nc = get_nc(P, n, features.shape[1], NUM_CORES)
    in_maps = make_in_maps(adj, features, W, b, P, NUM_CORES)
    res = run_bass_kernel_spmd(nc, in_maps, core_ids=list(range(NUM_CORES)))
    outs = [np.asarray(res.results[c]["out"], dtype=np.float32)
            for c in range(NUM_CORES)]
    return np.concatenate(outs, axis=0)


# revision 23
# speedup vs baseline: 1.0592x; 1.0592x over previous
"""GCN layer (normalized adjacency @ features -> linear -> relu) on 8 TRN2 NeuronCores.

Strategy (row-sharded, 1D node partition), v2:
  - Host shards adj by rows (P=1024 rows/core), adds the identity diagonal,
    TRANSPOSES the shard and stores it as fp8e4 (0/1/2 values are exact in
    e4m3, so this is lossless): 8MB/core of HBM traffic instead of 32MB, and
    no on-chip transpose at all. Tiles are pre-shuffled on the host so every
    DMA reads 4KB contiguous per partition line.
  - Rowsums: ones-vector matmuls on the PE against the fp8 tiles as they
    land (contraction over the partition dim = adjacency columns). The adj
    columns are split into two i-groups so gather#0 (own rows 0..511) fires
    at ~50% of streaming and hides under the rest.
  - Two AllGathers of the per-group rowsums; d = rsqrt via Sqrt + reciprocal
    + one Newton step. Own-row d is computed locally (no gather needed).
  - Main matmul: out_pre.T accumulated over 64 j-stripes with mixed-dtype
    matmuls (stationary = d-scaled bf16 features, moving = fp8 adj.T), then
    a small fp32 matmul with W.T, per-row d scale, bias, relu; per-stripe
    output DMA. Host concatenates the 8 [P,128] outputs.
"""

import numpy as np
import ml_dtypes

import concourse.bass as bass
import concourse.bacc as bacc
import concourse.mybir as mybir
import concourse.tile as tile
from concourse.bass_utils import run_bass_kernel_spmd

F32 = mybir.dt.float32
BF16 = mybir.dt.bfloat16
FP8 = mybir.dt.float8e4

N_FULL = 8192
F_DIM = 128
NUM_CORES = 8


def build_kernel(P=1024, N=8192, F=128, num_cores=8):
    """Build the SPMD Bass program. P = rows per core; N = total nodes."""
    assert P % 256 == 0 and N % 1024 == 0 and F == 128
    n_st = P // 128            # own-row stripes per core (8)
    n_jb = N // 128            # j-stripes globally (64)
    GB = N // 1024             # at-tile count per pass (j blocks of 1024)
    GC = P // 2                # columns per i-group (512)
    spg = n_st // 2            # own-row stripes per group (4)
    n_ts = N // n_st           # feature stripes (64) == n_jb

    nc = bacc.Bacc("TRN2", target_bir_lowering=False, debug=False,
                   num_devices=num_cores)

    # host-preshuffled adj.T: [g, b, p, jj, i] with j = b*1024 + jj*128 + p,
    # i = g*GC + i_local. Per (g,b,p): 8*GC bytes contiguous.
    at_h = nc.declare_dram_parameter("at8", [2, GB, 128, 8, GC], FP8,
                                     isOutput=False)
    ones8_h = nc.declare_dram_parameter("ones8", [128, 2, 128], FP8,
                                        isOutput=False)
    feat_h = nc.declare_dram_parameter("feat16", [128, n_jb, F], BF16,
                                       isOutput=False)
    w_h = nc.declare_dram_parameter("w", [F, F], F32, isOutput=False)
    bias_h = nc.declare_dram_parameter("bias_b", [128, F], F32, isOutput=False)
    eye32_h = nc.declare_dram_parameter("eye32", [128, 128], F32, isOutput=False)
    out_h = nc.declare_dram_parameter("out", [P, F], F32, isOutput=True)

    r_loc = [nc.dram_tensor(f"r_local{g}", [1, GC], F32) for g in range(2)]
    r_ful = [nc.dram_tensor(f"r_full{g}", [num_cores, GC], F32,
                            addr_space="Shared") for g in range(2)]


    at_ap = at_h.ap().rearrange("g b p jj i -> p g b jj i")
    out_ap = out_h.ap().rearrange("(s p) f -> p s f", p=128)

    with tile.TileContext(nc) as tc:
        with tc.tile_pool(name="const", bufs=1) as cpool, \
             tc.tile_pool(name="atp", bufs=1) as atp, \
             tc.tile_pool(name="p2", bufs=1) as p2, \
             tc.tile_pool(name="psR", bufs=1, space="PSUM") as psR, \
             tc.tile_pool(name="psM", bufs=1, space="PSUM") as psM, \
             tc.tile_pool(name="psT", bufs=1, space="PSUM") as psT:

            eye32 = cpool.tile([128, 128], F32)
            nc.scalar.dma_start(eye32, eye32_h[:])
            w_sb = cpool.tile([128, F], F32)
            nc.scalar.dma_start(w_sb, w_h[:])
            bias_bc = cpool.tile([128, F], F32)
            nc.scalar.dma_start(bias_bc, bias_h[:])
            feat16 = cpool.tile([128, n_jb, F], BF16)

            ones8 = cpool.tile([128, 2, 128], FP8)
            nc.scalar.dma_start(ones8, ones8_h[:])
            # pre-warm the Sqrt activation table set so the d = rsqrt(r)
            # chains don't pay the ~2.7us table load
            warm = cpool.tile([1, 1], F32)
            nc.scalar.activation(warm, eye32[0:1, 0:1],
                                 mybir.ActivationFunctionType.Sqrt)

            # PSUM residents
            pm = [psM.tile([128, GC], F32, tag=f"pm{h}", name=f"pm{h}")
                  for h in range(2)]
            down = psT.tile([128, n_st], F32, tag="down")

            at_tiles = [[None] * GB, [None] * GB]
            rs = cpool.tile([1, P], F32, name="rs")

            # ---- phase 1: stream adj.T, accumulate rowsums on the PE ----
            # rowsums via fp8 DoubleRow (2 j-stripes per matmul)
            for g in range(2):
                rsum = psR.tile([128, GC], F32, tag="rsum", name="rsum")
                for b in range(GB):
                    at = atp.tile([128, 8, GC], FP8, tag=f"at{g}{b}",
                                  name="at")
                    nc.sync.dma_start(at, at_ap[:, g, b])
                    at_tiles[g][b] = at
                    for jj in range(0, 8, 2):
                        nc.tensor.matmul(
                            rsum, lhsT=ones8, rhs=at[:, jj:jj + 2, :],
                            perf_mode=mybir.MatmulPerfMode.DoubleRow,
                            start=(b == 0 and jj == 0),
                            stop=(b == GB - 1 and jj == 6))
                nc.scalar.copy(rs[0:1, g * GC:(g + 1) * GC], rsum[0:1, :])
                nc.scalar.dma_start(r_loc[g][:], rs[0:1, g * GC:(g + 1) * GC])
                nc.gpsimd.collective_compute(
                    "AllGather", mybir.AluOpType.bypass,
                    replica_groups=[list(range(num_cores))],
                    ins=[r_loc[g][:].opt()],
                    outs=[r_ful[g][:].opt()],
                )
                if g == 0:
                    # feature load deferred: keeps pass-0 HBM bandwidth and
                    # the scalar ring free for the rowsum flush
                    nc.scalar.dma_start(feat16, feat_h[:])

            # ---- phase 2: d, df, main matmul, epilogue ----
            def rsqrt_newton(r_in, width, nm):
                sq = p2.tile([128, width], F32, tag=f"sq{nm}", name=f"sq{nm}")
                nc.scalar.activation(sq, r_in,
                                     mybir.ActivationFunctionType.Sqrt)
                y0 = p2.tile([128, width], F32, tag=f"y0{nm}", name=f"y0{nm}")
                nc.vector.reciprocal(y0, sq)
                yy = p2.tile([128, width], F32, tag=f"yy{nm}", name=f"yy{nm}")
                nc.vector.tensor_mul(yy, y0, y0)
                ryy = p2.tile([128, width], F32, tag=f"ry{nm}", name=f"ryy{nm}")
                nc.vector.tensor_mul(ryy, yy, r_in)
                corr = p2.tile([128, width], F32, tag=f"co{nm}", name=f"corr{nm}")
                nc.vector.tensor_scalar(out=corr, in0=ryy, scalar1=-0.5,
                                        scalar2=1.5,
                                        op0=mybir.AluOpType.mult,
                                        op1=mybir.AluOpType.add)
                d = p2.tile([128, width], F32, tag=f"d{nm}", name=f"d{nm}")
                nc.vector.tensor_mul(d, y0, corr)
                return d

            # W.T (stationary for the per-stripe linear) + own-row d: no
            # gather dependency, fills the gather wait window
            for s in range(n_st):
                nc.tensor.matmul(
                    down[:, s:s + 1],
                    lhsT=rs[0:1, s * 128:(s + 1) * 128],
                    rhs=eye32[0:1, 0:1], start=True, stop=True)
            pw = psT.tile([128, F], F32, tag="pp", bufs=2, name="pw")
            nc.tensor.matmul(pw, lhsT=w_sb, rhs=eye32, start=True, stop=True)
            wt_sb = cpool.tile([128, F], F32)
            nc.scalar.copy(wt_sb, pw)
            d_own = rsqrt_newton(down, n_st, "o")

            opre = p2.tile([128, P], F32)
            out_sb = p2.tile([128, n_st, F], F32)

            def d_for_group(g):
                rows = num_cores * spg
                rf = p2.tile([rows, 128], F32, tag="rf", bufs=2, name="rf")
                nc.sync.dma_start(rf, r_ful[g].ap().rearrange(
                    "c (s p) -> (c s) p", p=128))
                prT = psT.tile([128, rows], F32, tag="prT", name="prT")
                nc.tensor.matmul(prT, lhsT=rf, rhs=eye32[0:rows, 0:rows],
                                 start=True, stop=True)
                return rsqrt_newton(prT, rows, f"g{g}")

            def make_dfs(g, d_g):
                # dfs fed from two engines; every tile stays live so the
                # producers sprint ahead of the PE (no WAR throttling)
                dfs = []
                for c in range(num_cores):
                    for s in range(spg):
                        t = c * n_st + g * spg + s
                        idx = c * spg + s
                        df = p2.tile([128, F], BF16, tag="df", bufs=n_jb,
                                     name="df")
                        dcol = d_g[:, idx:idx + 1]
                        if idx % 2 == 0:
                            nc.scalar.mul(df, feat16[:, t, :], dcol)
                        else:
                            nc.vector.tensor_scalar(
                                out=df, in0=feat16[:, t, :], scalar1=dcol,
                                scalar2=None, op0=mybir.AluOpType.mult)
                        dfs.append((t, df))
                return dfs

            def finish_chunk(hc):
                nc.scalar.copy(opre[:, hc * GC:(hc + 1) * GC], pm[hc])
                for s in range(hc * spg, (hc + 1) * spg):
                    sl = s - hc * spg
                    p2m = psT.tile([128, F], F32, tag="pp", bufs=2,
                                   name="p2m")
                    nc.tensor.matmul(
                        p2m, lhsT=opre[:, (hc * spg + sl) * 128:
                                       (hc * spg + sl + 1) * 128],
                        rhs=wt_sb, start=True, stop=True)
                    epi = p2.tile([128, F], F32, tag="epi", bufs=2,
                                  name="epi")
                    nc.vector.scalar_tensor_tensor(
                        out=epi, in0=p2m, scalar=d_own[:, s:s + 1],
                        in1=bias_bc, op0=mybir.AluOpType.mult,
                        op1=mybir.AluOpType.add)
                    nc.vector.tensor_scalar_max(out_sb[:, s, :], epi, 0.0)
                    nc.scalar.dma_start(out_ap[:, s, :], out_sb[:, s, :])

            # group 0 (after gather#0): both chunks, stripe-major
            started = [False, False]
            d_g0 = d_for_group(0)
            dfs0 = make_dfs(0, d_g0)
            for t, df in dfs0:
                for hc in range(2):
                    nc.tensor.matmul(
                        pm[hc], lhsT=df,
                        rhs=at_tiles[hc][t // 8][:, t % 8, :],
                        start=not started[hc], stop=False)
                    started[hc] = True

            # group 1 (after gather#1): chunk-major so chunk 0's epilogue
            # overlaps chunk 1's matmuls
            d_g1 = d_for_group(1)
            dfs1 = make_dfs(1, d_g1)
            for hc in range(2):
                for idx, (t, df) in enumerate(dfs1):
                    nc.tensor.matmul(
                        pm[hc], lhsT=df,
                        rhs=at_tiles[hc][t // 8][:, t % 8, :],
                        start=False, stop=(idx == len(dfs1) - 1))
                finish_chunk(hc)

    nc.compile()
    return nc


def make_in_maps(adj, features, W, b, P, num_cores):
    """Shard + pre-transpose + fp8-encode inputs on the host."""
    adj = np.asarray(adj, dtype=np.float32)
    features = np.asarray(features, dtype=np.float32)
    W = np.asarray(W, dtype=np.float32)
    b = np.asarray(b, dtype=np.float32)
    N = adj.shape[0]
    GB = N // 1024
    GC = P // 2
    eye32 = np.eye(128, dtype=np.float32)
    bias_b = np.broadcast_to(b[None, :], (128, b.shape[0])).copy()
    feat16 = np.ascontiguousarray(
        features.astype(ml_dtypes.bfloat16)
        .reshape(N // 128, 128, features.shape[1]).transpose(1, 0, 2))
    in_maps = []
    idx = np.arange(P)
    for c in range(num_cores):
        shT = adj[c * P:(c + 1) * P, :].T.astype(ml_dtypes.float8_e4m3)
        # add the identity diagonal (0/1 -> 1/2; exact in e4m3)
        diag = shT[c * P + idx, idx].astype(np.float32) + 1.0
        shT[c * P + idx, idx] = diag.astype(ml_dtypes.float8_e4m3)
        # [N, P] -> [g, b, p, jj, i]
        at8 = np.ascontiguousarray(
            shT.reshape(GB, 8, 128, 2, GC).transpose(3, 0, 2, 1, 4))
        in_maps.append({
            "at8": at8,
            "ones8": np.ones((128, 2, 128), dtype=ml_dtypes.float8_e4m3),
            "feat16": feat16,
            "w": W,
            "bias_b": bias_b,
            "eye32": eye32,
        })
    return in_maps


_NC_CACHE = {}


def get_nc(P=N_FULL // NUM_CORES, N=N_FULL, F=F_DIM, num_cores=NUM_CORES):
    key = (P, N, F, num_cores)
    if key not in _NC_CACHE:
        _NC_CACHE[key] = build_kernel(P, N, F, num_cores)
    return _NC_CACHE[key]


def kernel(**inputs):
    adj = np.asarray(inputs["adj"], dtype=np.float32)
    features = np.asarray(inputs["features"], dtype=np.float32)
    W = np.asarray(inputs["W"], dtype=np.float32)
    b = np.asarray(inputs["b"], dtype=np.float32)
    n = adj.shape[0]
    P = n // NUM_CORES
    nc = get_nc(P, n, features.shape[1], NUM_CORES)
    in_maps = make_in_maps(adj, features, W, b, P, NUM_CORES)
    res = run_bass_kernel_spmd(nc, in_maps, core_ids=list(range(NUM_CORES)))
    outs = [np.asarray(res.results[c]["out"], dtype=np.float32)
            for c in range(NUM_CORES)]
    return np.concatenate(outs, axis=0)
